# revision 1
# baseline (speedup 1.0000x reference)
"""Multi-head attention block on 8 Trainium2 NeuronCores.

Problem: x[8,1024,768] -> qkv = x@w_qkv+b_qkv -> 12-head attention -> proj.
Sharding: pure data-parallel over batch (B=8 -> 1 batch element per core).
No collectives needed.

Per-core design (tokens n=1024, features d=768, heads h=12, hd=64):
  - x^T [768,1024] via PE transpose (fp32 has no DMA transpose)
  - v = x @ w_qkv[:,1536:] + b in natural [token, feature] layout
    (x^T slices as the stationary operand), stored fp16 with an extra
    ones column per (m-tile, head) -> [v | 1]
  - per head-pair hp (heads 2hp / 2hp+1):
      q^T,k^T f-tiles hp and 6+hp of (x @ w_qkv)^T: w_qkv tiles as
      stationary, x^T moving; per-head slices are [64, 1024] at
      partition base (h%2)*64
      scores^T[m,n] = k^T-slice.T @ q^T-slice: K=64 matmuls, the two
      heads alternate PE row groups 0/64 and run concurrently
      P = exp(scores/8) on ACT (no max subtraction: |scores| < ~8), fp16
      attnv: out^T rows 0..63 + softmax-denominator row 64 via [v | 1]
      normalize: DVE reciprocal + gpsimd partition_broadcast + DVE mult
    The pair loop is software-pipelined: pair hp's qk/scores run on PE
    while pair hp-1's attnv waits for its exps on ACT (ACT is the pacing
    engine in steady state).
  - proj: wa^T slices stationary, w_proj moving -> out in natural [n, d]
    layout, no final transpose.
Matmuls run as float32r (full PE rate, walrus requires producers to
round to f32r); P/v in fp16.
"""

import numpy as np

import concourse.bass as bass
import concourse.mybir as mybir
from concourse import bacc
from concourse.tile import TileContext
from concourse.bass_utils import run_bass_kernel_spmd
from concourse.masks import make_identity

P = 128
N = 1024          # tokens per batch element
D = 768           # model dim
H = 12            # heads
HD = 64           # head dim
KT = D // P       # 6 k-tiles over model dim
NT = N // P       # 8 token tiles
NCORES = 8
SCALE = HD ** -0.5  # 0.125

F32 = mybir.dt.float32
F32R = mybir.dt.float32r
BF16 = mybir.dt.bfloat16
F16 = mybir.dt.float16


def _emit(nc, reps=1):
    x = nc.dram_tensor("x", [N, D], F32, kind="ExternalInput")
    w_qkv = nc.dram_tensor("w_qkv", [D, 3 * D], F32, kind="ExternalInput")
    b_qkv = nc.dram_tensor("b_qkv", [3 * D], F32, kind="ExternalInput")
    w_proj = nc.dram_tensor("w_proj", [D, D], F32, kind="ExternalInput")
    b_proj = nc.dram_tensor("b_proj", [D], F32, kind="ExternalInput")
    out = nc.dram_tensor("out", [N, D], F32, kind="ExternalOutput")

    with TileContext(nc) as tc:
      for _rep in range(reps):
        with tc.tile_pool(name="main", bufs=1) as main, \
             tc.tile_pool(name="outbuf", bufs=2) as outbuf:
            v_sb = main.tile([P, NT, H, HD + 1], F16)   # v + ones column
            wa_sb = main.tile([P, KT, N], F32R)          # normalized attn out ^T
            bq_sb = main.tile([P, 2 * KT], F32)          # q,k bias (per partition)
            vb_sb = main.tile([P, D], F32)               # v bias (bcast over partitions)
            pb_sb = main.tile([P, D], F32)               # proj bias (bcast)
            ident = main.tile([P, P], F32)

            make_identity(nc, ident[:])
            nc.gpsimd.memset(v_sb[:, :, :, HD:HD + 1], 1.0)
            nc.gpsimd.dma_start(bq_sb[:], b_qkv[0:2 * D].rearrange("(o p) -> p o", p=P))
            nc.gpsimd.dma_start(vb_sb[:], b_qkv[2 * D:3 * D].unsqueeze(0).partition_broadcast(P))
            nc.gpsimd.dma_start(pb_sb[:], b_proj[:].unsqueeze(0).partition_broadcast(P))

            with tc.tile_pool(name="xt", bufs=1) as xtp, \
                 tc.tile_pool(name="wp", bufs=1) as wpp:
                xT = xtp.tile([P, KT, N], F32R)
                wp_sb = wpp.tile([P, KT, D], F32R)
                for kt in range(KT):
                    nc.gpsimd.dma_start(wp_sb[:, kt, :], w_proj[kt * P:(kt + 1) * P, :].bitcast(F32R))

                # ---- Phase A: load x, PE-transpose to x^T ----
                with tc.tile_pool(name="xload", bufs=3) as xlp, \
                     tc.tile_pool(name="tpsum", bufs=4, space="PSUM") as tpp:
                    for nt in range(NT):
                        xt = xlp.tile([P, D], F32)
                        nc.sync.dma_start(xt[:], x[nt * P:(nt + 1) * P, :])
                        for kt in range(KT):
                            pst = tpp.tile([P, P], F32, tag="tp", name=f"tp_{nt}_{kt}")
                            nc.tensor.transpose(pst[:], xt[:, kt * P:(kt + 1) * P], ident[:])
                            nc.vector.tensor_copy(xT[:, kt, nt * P:(nt + 1) * P], pst[:])

                # ---- Phase C: qk + attention, pipelined over head pairs ----
                with tc.tile_pool(name="wqk", bufs=12) as wqkp, \
                     tc.tile_pool(name="wv", bufs=1) as wvp, \
                     tc.tile_pool(name="qk", bufs=2) as qkp, \
                     tc.tile_pool(name="p", bufs=3) as ppool, \
                     tc.tile_pool(name="stat", bufs=2) as statp, \
                     tc.tile_pool(name="qkpsum", bufs=1, space="PSUM") as qpp, \
                     tc.tile_pool(name="spsum", bufs=2, space="PSUM") as spp, \
                     tc.tile_pool(name="opsum", bufs=2, space="PSUM") as opp:

                    wv_sb = wvp.tile([P, KT, D], F32R)
                    for kt in range(KT):
                        nc.gpsimd.dma_start(wv_sb[:, kt, :], w_qkv[kt * P:(kt + 1) * P, 2 * D:3 * D].bitcast(F32R))

                    def emit_v():
                        # v projection in natural [token, feature] layout;
                        # shares the opsum pool slots (runs before any attnv)
                        for nt in range(NT):
                            for c2 in range(2):
                                fs = slice(c2 * 384, (c2 + 1) * 384)
                                psv = opp.tile([P, 384], F32, tag="opsum", name=f"vpsum_{nt}_{c2}")
                                for kt in range(KT):
                                    nc.tensor.matmul(psv[:], xT[:, kt, nt * P:(nt + 1) * P],
                                                     wv_sb[:, kt, fs],
                                                     start=(kt == 0), stop=(kt == KT - 1))
                                nc.vector.tensor_add(
                                    v_sb[:, nt, c2 * 6:(c2 + 1) * 6, 0:HD],
                                    psv[:].rearrange("p (h d) -> p h d", d=HD),
                                    vb_sb[:, fs].rearrange("p (h d) -> p h d", d=HD))

                    def emit_qk(hp):
                        # q^T f-tile hp and k^T f-tile 6+hp for this pair
                        qk_t = qkp.tile([P, 2, N], F32R, tag="qk", name=f"qk_{hp}")
                        for i, ft in enumerate((hp, 6 + hp)):
                            wts = []
                            for kt in range(KT):
                                wt = wqkp.tile([P, P], F32R, tag="wqk", name=f"w_{ft}_{kt}")
                                nc.sync.dma_start(wt[:], w_qkv[kt * P:(kt + 1) * P, ft * P:(ft + 1) * P].bitcast(F32R))
                                wts.append(wt)
                            ps = qpp.tile([P, N], F32, tag="qkpsum", name=f"qkps_{ft}")
                            for ch in range(2):
                                cs = slice(ch * 512, (ch + 1) * 512)
                                for kt in range(KT):
                                    nc.tensor.matmul(ps[:, cs], wts[kt][:], xT[:, kt, cs],
                                                     start=(kt == 0), stop=(kt == KT - 1))
                            nc.vector.tensor_add(qk_t[:, i, :], ps[:],
                                                 bq_sb[:, ft:ft + 1].to_broadcast([P, N]))
                        return qk_t

                    def emit_scores(hp, qk_t, ptiles):
                        # the two heads alternate PE row groups 0/64
                        heads = (2 * hp, 2 * hp + 1)
                        for mt in range(NT):
                            spss = {
                                h: spp.tile([P, N], F32, tag="spsum", name=f"spsum_{h}_{mt}")
                                for h in heads
                            }
                            for ch in range(2):
                                cs = slice(ch * 512, (ch + 1) * 512)
                                for h in heads:
                                    base = (h % 2) * HD
                                    nc.tensor.matmul(
                                        spss[h][:, cs],
                                        qk_t[base:base + HD, 1, mt * P:(mt + 1) * P],
                                        qk_t[base:base + HD, 0, cs],
                                        start=True, stop=True)
                            for h in heads:
                                nc.scalar.activation(ptiles[h][:, mt, :], spss[h][:],
                                                     mybir.ActivationFunctionType.Exp,
                                                     scale=SCALE)

                    def emit_attnv(hp, ptiles):
                        for h in (2 * hp, 2 * hp + 1):
                            base = (h % 2) * HD
                            psos = []
                            s0 = statp.tile([1, N], F32, tag="s0", name=f"s0_{h}")
                            rb = statp.tile([HD, N], F32, tag="rb", name=f"rb_{h}")
                            for ch in range(2):
                                cs = slice(ch * 512, (ch + 1) * 512)
                                pso = opp.tile([HD + 1, 512], F32, tag="opsum", name=f"opsum_{h}_{ch}")
                                for mt in range(NT):
                                    nc.tensor.matmul(pso[:], v_sb[:, mt, h, :],
                                                     ptiles[h][:, mt, cs],
                                                     start=(mt == 0), stop=(mt == NT - 1))
                                nc.vector.tensor_copy(s0[0:1, cs], pso[HD:HD + 1, :])
                                psos.append(pso)
                            nc.vector.reciprocal(s0[:], s0[:])
                            nc.gpsimd.partition_broadcast(rb[:], s0[:])
                            for ch in range(2):
                                cs = slice(ch * 512, (ch + 1) * 512)
                                nc.vector.tensor_mul(wa_sb[base:base + HD, h // 2, cs],
                                                     psos[ch][0:HD, :], rb[:, cs])

                    prev = None
                    for hp in range(H // 2):
                        qk_t = emit_qk(hp)
                        ptiles = {
                            h: ppool.tile([P, NT, N], F16, tag="p", name=f"p_{h}")
                            for h in (2 * hp, 2 * hp + 1)
                        }
                        emit_scores(hp, qk_t, ptiles)
                        if hp == 0:
                            # v projection overlaps pair 0's exps on ACT
                            emit_v()
                        if prev is not None:
                            emit_attnv(prev[0], prev[1])
                        prev = (hp, ptiles)
                    emit_attnv(prev[0], prev[1])

                # ---- Phase D: output projection ----
                with tc.tile_pool(name="prpsum", bufs=4, space="PSUM") as prp:
                    for nt in range(NT):
                        ot = outbuf.tile([P, D], F32, tag="out", name=f"out_{nt}")
                        for jc in range(2):
                            js = slice(jc * 384, (jc + 1) * 384)
                            psp = prp.tile([P, 384], F32, tag="prpsum", name=f"prps_{nt}_{jc}")
                            for kt in range(KT):
                                nc.tensor.matmul(psp[:], wa_sb[:, kt, nt * P:(nt + 1) * P],
                                                 wp_sb[:, kt, js],
                                                 start=(kt == 0), stop=(kt == KT - 1))
                            nc.vector.tensor_add(ot[:, js], psp[:], pb_sb[:, js])
                        nc.sync.dma_start(out[nt * P:(nt + 1) * P, :], ot[:])


def build(reps=1):
    nc = bacc.Bacc("TRN2", target_bir_lowering=False, debug=False, num_devices=NCORES)
    _emit(nc, reps=reps)
    nc.compile()
    return nc


_CACHE = {}


def _get_nc():
    if "nc" not in _CACHE:
        _CACHE["nc"] = build()
    return _CACHE["nc"]


def kernel(x, w_qkv, b_qkv, w_proj, b_proj):
    x = np.ascontiguousarray(np.asarray(x, dtype=np.float32))
    w_qkv = np.ascontiguousarray(np.asarray(w_qkv, dtype=np.float32))
    b_qkv = np.ascontiguousarray(np.asarray(b_qkv, dtype=np.float32))
    w_proj = np.ascontiguousarray(np.asarray(w_proj, dtype=np.float32))
    b_proj = np.ascontiguousarray(np.asarray(b_proj, dtype=np.float32))

    nc = _get_nc()
    in_maps = [
        {"x": np.ascontiguousarray(x[c]), "w_qkv": w_qkv, "b_qkv": b_qkv,
         "w_proj": w_proj, "b_proj": b_proj}
        for c in range(NCORES)
    ]
    res = run_bass_kernel_spmd(nc, in_maps, list(range(NCORES)))
    return np.stack([res.results[c]["out"] for c in range(NCORES)], axis=0)



# revision 4
# speedup vs baseline: 1.4993x; 1.4993x over previous
"""Multi-head attention block on 8 Trainium2 NeuronCores — v2.

Problem: x[8,1024,768] -> qkv = x@w_qkv+b_qkv -> 12-head attention -> proj.
Sharding: pure data-parallel over batch (B=8 -> 1 batch element per core).
No collectives.

v2 design (per core: tokens n=1024, d=768, h=12, hd=64):
  - All matmul operands fp16: inputs cast f32->f16 in-flight by gpsimd
    (software-DGE) cast DMAs; x^T produced by XBAR DMA transpose (16-bit)
    instead of PE transposes + DVE copies.
  - qk: w tiles stationary, x^T moving -> q^T,k^T per head pair, fp16.
  - scores^T = k^T-slice.T @ q^T: two heads of a pair run concurrently on
    PE row groups 0/64. exp on ACT (pacing engine), fp16 P tiles.
  - attnv: the two heads run CONCURRENTLY via col-tiled matmuls
    (v_h0 -> array cols 0-63, v_h1 -> cols 64-127, each streaming its own
    P^T chunk) — 2x over the serial ones-column scheme.
  - softmax denominators: 4-way col-tiled ones[128,32] matmuls, one col
    group per (head, query-chunk), accumulated over key tiles. Output rows
    are 32-wide pre-broadcast sums; one DVE reciprocal [128,512] per pair
    then 32-row tensor_muls normalize into wa fp16.
  - proj: wa slices stationary, w_proj moving -> natural [n, d] output.
PSUM: scores 2x[128,1024] + qk 1x[128,1024] + attnv 1x[128,512] +
den 1x[128,512] = 8 banks exactly.
"""

import numpy as np

import concourse.bass as bass
import concourse.mybir as mybir
from concourse import bacc
from concourse.tile import TileContext
from concourse.bass_utils import run_bass_kernel_spmd

P = 128
N = 1024          # tokens per batch element
D = 768           # model dim
H = 12            # heads
HD = 64           # head dim
KT = D // P       # 6 k-tiles over model dim
NT = N // P       # 8 token tiles
NCORES = 8
SCALE = HD ** -0.5  # 0.125

F32 = mybir.dt.float32
F16 = mybir.dt.float16


def _emit(nc, reps=1):
    x = nc.dram_tensor("x", [N, D], F32, kind="ExternalInput")
    w_qkv = nc.dram_tensor("w_qkv", [D, 3 * D], F32, kind="ExternalInput")
    b_qkv = nc.dram_tensor("b_qkv", [3 * D], F32, kind="ExternalInput")
    w_proj = nc.dram_tensor("w_proj", [D, D], F32, kind="ExternalInput")
    b_proj = nc.dram_tensor("b_proj", [D], F32, kind="ExternalInput")
    out = nc.dram_tensor("out", [N, D], F32, kind="ExternalOutput")
    x16 = nc.dram_tensor("x16_scratch", [N, D], F16, kind="Internal")

    with TileContext(nc) as tc:
      for _rep in range(reps):
        with tc.tile_pool(name="main", bufs=1) as main, \
             tc.tile_pool(name="outbuf", bufs=2) as outbuf:
            xT = main.tile([P, KT, N], F16)
            wqk = main.tile([P, KT, 2 * D], F16)
            wv = main.tile([P, KT, D], F16)
            wp = main.tile([P, KT, D], F16)
            v_sb = main.tile([P, NT, H, HD], F16)
            wa = main.tile([P, KT, N], F16)
            bqk = main.tile([P, 2 * KT], F32)
            vb = main.tile([P, D], F32)
            pb = main.tile([P, D], F32)
            ones32 = main.tile([P, 32], F16)

            nc.gpsimd.memset(ones32[:], 1.0)
            nc.gpsimd.dma_start(bqk[:], b_qkv[0:2 * D].rearrange("(o p) -> p o", p=P))
            nc.gpsimd.dma_start(vb[:], b_qkv[2 * D:3 * D].unsqueeze(0).partition_broadcast(P))
            nc.gpsimd.dma_start(pb[:], b_proj[:].unsqueeze(0).partition_broadcast(P))
            for kt in range(KT):
                nc.gpsimd.dma_start(wqk[:, kt, :], w_qkv[kt * P:(kt + 1) * P, 0:2 * D])
                nc.gpsimd.dma_start(wv[:, kt, :], w_qkv[kt * P:(kt + 1) * P, 2 * D:3 * D])
                nc.gpsimd.dma_start(wp[:, kt, :], w_proj[kt * P:(kt + 1) * P, :])

            # ---- Phase A: x --(cast DMA)--> x16 DRAM, then one big XBAR
            # DMA transpose per kt: [1024, 128] DRAM -> [128, 1024] SBUF.
            # (48 small SBUF transposes cost ~1.3us of queue time EACH; 6 big
            # ones amortize the per-DMA cost.)
            for nt in range(2):
                nc.gpsimd.dma_start(x16[nt * 512:(nt + 1) * 512, :],
                                    x[nt * 512:(nt + 1) * 512, :])
            for kt in range(KT):
                nc.sync.dma_start(xT[:, kt, :],
                                  x16[:, kt * P:(kt + 1) * P], transpose=True)

            # ---- Phase C: qkv + attention, pipelined over head pairs ----
            with tc.tile_pool(name="qk", bufs=2) as qkp, \
                 tc.tile_pool(name="p", bufs=2) as ppool, \
                 tc.tile_pool(name="rb", bufs=2) as rbp, \
                 tc.tile_pool(name="qkpsum", bufs=1, space="PSUM") as qpp, \
                 tc.tile_pool(name="spsum", bufs=2, space="PSUM") as spp, \
                 tc.tile_pool(name="avpsum", bufs=1, space="PSUM") as avp, \
                 tc.tile_pool(name="denpsum", bufs=1, space="PSUM") as dnp:

                def vp_tile(idx, name):
                    # ping-pong small psum tiles across the av/den slots
                    pool, tag = (avp, "av") if idx % 2 == 0 else (dnp, "den")
                    return pool.tile([P, 512], F32, tag=tag, name=name)

                def emit_v():
                    for nt in range(NT):
                        for c2 in range(2):
                            fs = slice(c2 * 384, (c2 + 1) * 384)
                            psv = vp_tile(nt * 2 + c2, f"vps_{nt}_{c2}")
                            for kt in range(KT):
                                nc.tensor.matmul(psv[:, 0:384],
                                                 xT[:, kt, nt * P:(nt + 1) * P],
                                                 wv[:, kt, fs],
                                                 start=(kt == 0), stop=(kt == KT - 1))
                            nc.vector.tensor_add(
                                v_sb[:, nt, c2 * 6:(c2 + 1) * 6, :],
                                psv[:, 0:384].rearrange("p (h d) -> p h d", d=HD),
                                vb[:, fs].rearrange("p (h d) -> p h d", d=HD))

                def emit_qk(hp):
                    qk_t = qkp.tile([P, 2, N], F16, tag="qk", name=f"qk_{hp}")
                    for i, ft in enumerate((hp, 6 + hp)):
                        ps = qpp.tile([P, N], F32, tag="qkps", name=f"qkps_{ft}")
                        for ch in range(2):
                            cs = slice(ch * 512, (ch + 1) * 512)
                            for kt in range(KT):
                                nc.tensor.matmul(ps[:, cs],
                                                 wqk[:, kt, ft * P:(ft + 1) * P],
                                                 xT[:, kt, cs],
                                                 start=(kt == 0), stop=(kt == KT - 1))
                        nc.vector.tensor_add(qk_t[:, i, :], ps[:],
                                             bqk[:, ft:ft + 1].to_broadcast([P, N]))
                    return qk_t

                def emit_scores(hp, qk_t, ptile):
                    # two heads on PE row groups 0/64; exp batched per head
                    for mt in range(NT):
                        spss = [
                            spp.tile([P, N], F32, tag="sps", name=f"sps_{hp}_{mt}_{i}")
                            for i in range(2)
                        ]
                        for ch in range(2):
                            cs = slice(ch * 512, (ch + 1) * 512)
                            for i in range(2):
                                base = i * HD
                                nc.tensor.matmul(
                                    spss[i][:, cs],
                                    qk_t[base:base + HD, 1, mt * P:(mt + 1) * P],
                                    qk_t[base:base + HD, 0, cs],
                                    start=True, stop=True)
                        for i in range(2):
                            nc.scalar.activation(ptile[:, mt, i, :], spss[i][:],
                                                 mybir.ActivationFunctionType.Exp,
                                                 scale=SCALE)

                def emit_attnv(hp, ptile):
                    h0, h1 = 2 * hp, 2 * hp + 1
                    # denominators: 4-way col-tiled ones matmuls, one col group
                    # per (head, chunk), accumulated over key tiles
                    rb = rbp.tile([P, 512], F32, tag="rb", name=f"rb_{hp}")
                    dps = dnp.tile([P, 512], F32, tag="den", name=f"den_{hp}")
                    for mt in range(NT):
                        for i in range(2):
                            for ch in range(2):
                                j = 2 * i + ch
                                nc.tensor.matmul(
                                    dps[32 * j:32 * j + 32, :], ones32[:],
                                    ptile[:, mt, i, ch * 512:(ch + 1) * 512],
                                    start=(mt == 0), stop=(mt == NT - 1),
                                    tile_position=(0, 32 * j))
                    nc.vector.reciprocal(rb[:], dps[:])

                    for ch in range(2):
                        cs = slice(ch * 512, (ch + 1) * 512)
                        aps = avp.tile([P, 512], F32, tag="av", name=f"av_{hp}_{ch}")
                        if True:
                            for mt in range(NT):
                                for i in range(2):
                                    nc.tensor.matmul(aps[64 * i:64 * i + 64, :],
                                                     v_sb[:, mt, 2 * hp + i, :],
                                                     ptile[:, mt, i, cs],
                                                     start=(mt == 0), stop=(mt == NT - 1))
                        for i in range(2):
                            for b in range(2):
                                r0 = 64 * i + 32 * b
                                nc.vector.tensor_mul(
                                    wa[r0:r0 + 32, hp, cs],
                                    aps[r0:r0 + 32, :],
                                    rb[32 * (2 * i + ch):32 * (2 * i + ch) + 32, :])

                prev = None
                for hp in range(H // 2):
                    qk_t = emit_qk(hp)
                    ptile = ppool.tile([P, NT, 2, N], F16, tag="p", name=f"p_{hp}")
                    emit_scores(hp, qk_t, ptile)
                    if hp == 0:
                        emit_v()
                    if prev is not None:
                        emit_attnv(prev[0], prev[1])
                    prev = (hp, ptile)
                emit_attnv(prev[0], prev[1])

                # ---- Phase D: output projection ----
                for nt in range(NT):
                    ot = outbuf.tile([P, D], F32, tag="out", name=f"out_{nt}")
                    for jc in range(2):
                        js = slice(jc * 384, (jc + 1) * 384)
                        psp = vp_tile(nt * 2 + jc, f"prps_{nt}_{jc}")
                        for kt in range(KT):
                            nc.tensor.matmul(psp[:, 0:384],
                                             wa[:, kt, nt * P:(nt + 1) * P],
                                             wp[:, kt, js],
                                             start=(kt == 0), stop=(kt == KT - 1))
                        nc.vector.tensor_add(ot[:, js], psp[:, 0:384], pb[:, js])
                    nc.sync.dma_start(out[nt * P:(nt + 1) * P, :], ot[:])


def build(reps=1):
    nc = bacc.Bacc("TRN2", target_bir_lowering=False, debug=False, num_devices=NCORES)
    _emit(nc, reps=reps)
    nc.compile()
    return nc


_CACHE = {}


def _get_nc():
    if "nc" not in _CACHE:
        _CACHE["nc"] = build()
    return _CACHE["nc"]


def kernel(x, w_qkv, b_qkv, w_proj, b_proj):
    x = np.ascontiguousarray(np.asarray(x, dtype=np.float32))
    w_qkv = np.ascontiguousarray(np.asarray(w_qkv, dtype=np.float32))
    b_qkv = np.ascontiguousarray(np.asarray(b_qkv, dtype=np.float32))
    w_proj = np.ascontiguousarray(np.asarray(w_proj, dtype=np.float32))
    b_proj = np.ascontiguousarray(np.asarray(b_proj, dtype=np.float32))

    nc = _get_nc()
    in_maps = [
        {"x": np.ascontiguousarray(x[c]), "w_qkv": w_qkv, "b_qkv": b_qkv,
         "w_proj": w_proj, "b_proj": b_proj}
        for c in range(NCORES)
    ]
    res = run_bass_kernel_spmd(nc, in_maps, list(range(NCORES)))
    return np.stack([res.results[c]["out"] for c in range(NCORES)], axis=0)
